# revision 34
# baseline (speedup 1.0000x reference)
"""Causal self-attention (B=4, T=1024, C=1024, H=16) on 8 trn2 NeuronCores.

Sharding: core i handles batch b = i // 2 and head-group hg = i % 2
(8 heads = 512 of the 1024 channel dims). Each core computes

    qkv       = x[b] @ W_qkv[:, local]           (bf16 matmuls)
    P^T       = exp((k_h^T q_h) / 8) (causal)    (unstable softmax, bf16 P)
    y'^T      = [v_h | 1]^T @ P^T                (bf16, gives y^T + row-sums D)
    y^T       = y'^T / D                         (recip + bcast + DVE mul)
    partial   = y^T.T @ W_proj[local, :]         (bf16, bf16 output)

Host sums the two head-group partials per batch and adds b_proj.
b_qkv is all-zeros by construction (spec fill: zeros) so the qkv bias
add is elided on-device (host fallback guards the general case).

Weights are host-pretiled so every SBUF weight tile is one contiguous
DMA, all issued at the top of the program; the first x/wv chunks go on
the second HWDGE queue (qAct) split fine so the v-projection starts as
early as possible.

Pipeline per head pair hp (sts for hp+1, V groups for hp):
  [qk proj (hp+1) | PV g0,g1 (hp) | 12 S^T units (hp+1) | PV g2,g3 +
   norm chains (hp)]
with the out-projection after the last pair's groups. Engine budget per
iteration (PE 14.5us): ACT = 24 exps + 1 k-copy; DVE = q/k copies, 16
tri-mask muls, y'/D copies out of PSUM, reciprocals, normalize muls;
GpSimd = 4 partition-broadcasts; SP queue = input DMAs + D-row remaps +
odd-half yT staging; qAct queue = output DMAs.

Softmax denominator: the V matmul's ones-column lands row sums on PSUM
partition 64; DVE copies y'+D to SBUF (frees the PSUM bank), a DMA
remaps the D row to partition 0 (custom DVE reciprocal is base-0 only),
gpsimd broadcasts 1/D across partitions, DVE multiplies. Norm chains
are phase-split (all matmul+recip before bcast+mul) so no chain op ever
queues in front of the S->exp->mask->PV critical path; even halves
write yT directly, odd halves stage at base 0 and DMA-remap to
partitions 64-127 (DVE operands must share a partition base).

PSUM: 5-bank main pool (qk/S/out-proj) + 3-bank py pool (PV groups);
out-proj tiles alternate pools so 4 T-tiles can accumulate in flight
while the last head pair's norm chains drain. yT is split per (head
pair, query half) into 8 tiles to decouple out-proj reads from late
writers.
"""

import numpy as np
from contextlib import ExitStack

import ml_dtypes

import concourse.bacc as bacc
import concourse.tile as tile
import concourse.mybir as mybir
from concourse.bass_utils import run_bass_kernel_spmd
from concourse.masks import make_upper_triangular

B, T, C, H, HD = 4, 1024, 1024, 16, 64
NCORES = 8
HPG = 8            # heads per core
DL = HPG * HD      # 512 local channel dims per core
P = 128

F32 = mybir.dt.float32
BF16 = mybir.dt.bfloat16
EXP = mybir.ActivationFunctionType.Exp

PV = BF16
MMDT = BF16


def _build_program():
    nc = bacc.Bacc("TRN2", target_bir_lowering=False)

    xT = nc.dram_tensor("xT", [C, T], MMDT, kind="ExternalInput").ap()
    # host-pretiled: [p, cc, n] so each SBUF tile is one contiguous DMA
    wv = nc.dram_tensor("wv", [P, 8, DL], MMDT, kind="ExternalInput").ap()
    wq = nc.dram_tensor("wq", [4, P, 8, P], MMDT, kind="ExternalInput").ap()
    wk = nc.dram_tensor("wk", [4, P, 8, P], MMDT, kind="ExternalInput").ap()
    wp = nc.dram_tensor("wp", [P, 4, C], PV, kind="ExternalInput").ap()
    outp = nc.dram_tensor("outp", [T, C], PV, kind="ExternalOutput").ap()

    with tile.TileContext(nc) as tc:
        with ExitStack() as ctx:
            consts = ctx.enter_context(tc.tile_pool(name="consts", bufs=1))
            xt_pool = ctx.enter_context(tc.tile_pool(name="xt", bufs=8))
            w_pool = ctx.enter_context(tc.tile_pool(name="w", bufs=1))
            qk_pool = ctx.enter_context(tc.tile_pool(name="qk", bufs=2))
            v_pool = ctx.enter_context(tc.tile_pool(name="v", bufs=8))
            pt_pool = ctx.enter_context(tc.tile_pool(name="pt", bufs=52))
            yt_pool = ctx.enter_context(tc.tile_pool(name="yt", bufs=8))
            d_pool = ctx.enter_context(tc.tile_pool(name="d", bufs=12))
            out_pool = ctx.enter_context(tc.tile_pool(name="out", bufs=8))
            ps = ctx.enter_context(tc.tile_pool(name="ps", bufs=5, space="PSUM"))
            ps_py = ctx.enter_context(tc.tile_pool(name="psy", bufs=3, space="PSUM"))

            # ---- tiles ----
            xt = [xt_pool.tile([P, T], MMDT, name="xt") for _ in range(8)]
            wv_sb = w_pool.tile([P, 8, DL], MMDT, name="wv")
            wq_sb = [w_pool.tile([P, 8, P], MMDT, name="wq") for _ in range(4)]
            wk_sb = [w_pool.tile([P, 8, P], MMDT, name="wk") for _ in range(4)]
            wp_sb = w_pool.tile([P, 4, C], PV, name="wp")

            # ---- all input DMAs, just-in-time order ----
            # first chunks split fine + on the second HWDGE queue so the
            # v-projection can start as early as possible
            nc.scalar.dma_start(wv_sb[:, 0, :], wv[:, 0, :])
            nc.scalar.dma_start(xt[0][:, 0:256], xT[0:P, 0:256])
            nc.scalar.dma_start(xt[0][:, 256:512], xT[0:P, 256:512])
            nc.scalar.dma_start(xt[0][:, 512:1024], xT[0:P, 512:1024])
            nc.scalar.dma_start(wv_sb[:, 1, :], wv[:, 1, :])
            nc.scalar.dma_start(xt[1][:], xT[P : 2 * P, :])
            for cc in range(2, 8):
                nc.sync.dma_start(xt[cc][:], xT[P * cc : P * (cc + 1), :])
                nc.sync.dma_start(wv_sb[:, cc, :], wv[:, cc, :])
            nc.sync.dma_start(wq_sb[0][:], wq[0])
            nc.sync.dma_start(wk_sb[0][:], wk[0])
            nc.sync.dma_start(wp_sb[:], wp)
            for dt_ in range(1, 4):
                nc.sync.dma_start(wq_sb[dt_][:], wq[dt_])
                nc.sync.dma_start(wk_sb[dt_][:], wk[dt_])

            # ---- constants ----
            tri = consts.tile([P, P], PV, name="tri")  # 1 where tq >= s
            make_upper_triangular(nc, tri[:], val=1.0, diag=True)

            # PE warm-up: dummy matmuls on a zeroed tile while the first x
            # chunks stream in, so the PE clock is ramped to max before the
            # real v-projection starts (cold PE runs 2-4x slower for ~3us)
            warm = consts.tile([P, 512], PV, name="warm")
            nc.vector.memset(warm[:], 0.0)
            wps = ps.tile([P, 512], F32, name="ps")
            for _ in range(24):
                nc.tensor.matmul(
                    wps[:, 0:256],
                    lhsT=warm[:, 0:P],
                    rhs=warm[:, 0:256],
                    start=True,
                    stop=True,
                )

            # v tiles: [s=128, 8 heads x (64 dims + ones col)]
            v_sb = []
            for j in range(8):
                vt = v_pool.tile([P, HPG * (HD + 1)], PV, name="v")
                ones_cols = vt[:].rearrange("p (h e) -> p h e", e=HD + 1)[
                    :, :, HD : HD + 1
                ]
                nc.vector.memset(ones_cols, 1.0)
                v_sb.append(vt)

            qT_sb = [qk_pool.tile([P, 4, T], MMDT, name="qT") for _ in range(2)]
            kT_sb = [qk_pool.tile([P, 4, T], MMDT, name="kT") for _ in range(2)]
            # split per (head pair, query half) so late writers never gate
            # early out-proj reads through coarse dependency tracking
            yT_sb = [
                [yt_pool.tile([P, 512], PV, name="yT") for _ in range(2)]
                for _ in range(4)
            ]

            # ---- v projection (needed by every head pair), 2 waves ----
            for wave in range(2):
                ps_t = [ps.tile([P, 512], F32, name="ps") for _ in range(4)]
                for cc in range(8):
                    for wt_ in range(4):
                        tt = 4 * wave + wt_
                        nc.tensor.matmul(
                            ps_t[wt_][:, :],
                            lhsT=xt[cc][:, P * tt : P * (tt + 1)],
                            rhs=wv_sb[:, cc, :],
                            start=(cc == 0),
                            stop=(cc == 7),
                        )
                for wt_ in range(4):
                    tt = 4 * wave + wt_
                    out_ap = v_sb[tt][:].rearrange("p (h e) -> p h e", e=HD + 1)[
                        :, :, 0:HD
                    ]
                    in_ap = ps_t[wt_][:].rearrange("p (h e) -> p h e", e=HD)
                    nc.scalar.copy(out_ap, in_ap)

            def issue_qk(dt_):
                """q and k projections for head pair dt_ (128 channel dims)."""
                buf = dt_ % 2
                for w_sb, dest, cengs in (
                    (wq_sb[dt_], qT_sb[buf],
                     (nc.vector.tensor_copy, nc.vector.tensor_copy)),
                    (wk_sb[dt_], kT_sb[buf],
                     (nc.vector.tensor_copy, nc.scalar.copy)),
                ):
                    pst = [ps.tile([P, 512], F32, name="ps") for _ in range(2)]
                    for cc in range(8):
                        for tch in range(2):
                            nc.tensor.matmul(
                                pst[tch][:, :],
                                lhsT=w_sb[:, cc, :],
                                rhs=xt[cc][:, 512 * tch : 512 * (tch + 1)],
                                start=(cc == 0),
                                stop=(cc == 7),
                            )
                    for tch in range(2):
                        cengs[tch](
                            dest[:, dt_, 512 * tch : 512 * (tch + 1)], pst[tch][:, :]
                        )

            def emit_st_unit(hp, pts, c, j):
                """One S^T block (both halves) + exp + causal mask."""
                buf = hp % 2
                off = max(0, P * (j - 4 * c))
                n = 512 - off
                for half in range(2):
                    pr = 64 * half
                    pss = ps.tile([P, 512], F32, name="ps")
                    nc.tensor.matmul(
                        pss[:, :n],
                        lhsT=kT_sb[buf][pr : pr + 64, hp, P * j : P * (j + 1)],
                        rhs=qT_sb[buf][pr : pr + 64, hp, 512 * c + off : 512 * (c + 1)],
                        start=True,
                        stop=True,
                    )
                    pt = pt_pool.tile([P, 512], PV, name="pt")
                    nc.scalar.activation(
                        out=pt[:, off:512], in_=pss[:, :n], func=EXP, scale=0.125
                    )
                    if j >= 4 * c:
                        nc.vector.tensor_mul(
                            pt[:, off : off + P], pt[:, off : off + P], tri[:]
                        )
                    pts[(half, c, j)] = pt

            def emit_v_matmul(hp, pts, half, c):
                """V matmuls + D-row extract + reciprocal for one (half, c).

                DVE copies y'+D out of PSUM (freeing the bank), a sync-queue
                DMA remaps the D row to partition 0, DVE reciprocals it there
                (custom op, base-0 only). The bcast+mul live in emit_v_norm so
                recips of later groups never queue behind muls of earlier ones
                on DVE.
                """
                h = 2 * hp + half
                jmax = 4 * c + 3
                py = ps_py.tile([P, 512], F32, name="py")
                for j in range(jmax + 1):
                    off = max(0, P * (j - 4 * c))
                    nc.tensor.matmul(
                        py[0 : HD + 1, off:512],
                        lhsT=v_sb[j][:, (HD + 1) * h : (HD + 1) * (h + 1)],
                        rhs=pts[(half, c, j)][:, off:512],
                        start=(j == 0),
                        stop=(j == jmax),
                    )
                yc = d_pool.tile([HD + 1, 512], F32, name="yc")
                nc.vector.tensor_copy(yc[:], py[0 : HD + 1, :])
                d2 = d_pool.tile([P, 512], F32, name="d")
                nc.sync.dma_start(d2[0:1, :], yc[HD : HD + 1, :])
                # custom DVE op only at partition base 0 (HW quirk)
                nc.vector.reciprocal_approx_fast(d2[0:1, :], d2[0:1, :])
                return yc, d2

            def emit_v_norm(hp, pyd2, half, c):
                """Broadcast 1/D and multiply into yT (odd halves staged)."""
                yc, d2 = pyd2
                nc.gpsimd.partition_broadcast(d2[0:HD, :], d2[0:1, :])
                pr = 64 * half
                dst = yT_sb[hp][c][pr : pr + HD, :]
                if half == 0:
                    nc.vector.tensor_mul(dst, yc[0:HD, :], d2[0:HD, :])
                else:
                    stg = d_pool.tile([HD, 512], PV, name="stg")
                    nc.vector.tensor_mul(stg[:], yc[0:HD, :], d2[0:HD, :])
                    nc.sync.dma_start(dst, stg[:])

            def emit_out_proj(tts):
                for tt in tts:
                    pool, pnm = (ps, "ps") if tt % 2 == 0 else (ps_py, "py")
                    pouts = [pool.tile([P, 512], F32, name=pnm) for _ in range(2)]
                    for dc in range(4):
                        for cch in range(2):
                            nc.tensor.matmul(
                                pouts[cch][:, :],
                                lhsT=yT_sb[dc][tt // 4][
                                    :, P * (tt % 4) : P * (tt % 4 + 1)
                                ],
                                rhs=wp_sb[:, dc, 512 * cch : 512 * (cch + 1)],
                                start=(dc == 0),
                                stop=(dc == 3),
                            )
                    for cch in range(2):
                        ot = out_pool.tile([P, 512], PV, name="out")
                        if cch == 0:
                            nc.scalar.copy(ot[:], pouts[cch][:])
                        else:
                            nc.vector.tensor_copy(ot[:], pouts[cch][:])
                        nc.scalar.dma_start(
                            outp[P * tt : P * (tt + 1), 512 * cch : 512 * (cch + 1)],
                            ot[:],
                        )

            # ---- pipelined qk + attention ----
            # Per head pair: qk proj for the next pair, ALL 12 S^T units for
            # the next pair, then this pair's 4 V groups (c=0 halves first).
            # st units precede v groups on every engine stream, so the
            # normalization chain never blocks the S->PV critical path.
            st_order = [(c, j) for c in range(2) for j in range(4 * c + 4)]
            v_order = [(0, 0), (1, 0), (0, 1), (1, 1)]  # c=0 halves first
            issue_qk(0)
            pts_cur = {}
            for c, j in st_order:
                emit_st_unit(0, pts_cur, c, j)
            for hp in range(4):
                pts_next = {}
                if hp + 1 < 4:
                    issue_qk(hp + 1)
                    pyd = [emit_v_matmul(hp, pts_cur, *v_order[0])]
                    pyd.append(emit_v_matmul(hp, pts_cur, *v_order[1]))
                    for c, j in st_order[:6]:
                        emit_st_unit(hp + 1, pts_next, c, j)
                    emit_v_norm(hp, pyd[0], *v_order[0])
                    pyd.append(emit_v_matmul(hp, pts_cur, *v_order[2]))
                    for c, j in st_order[6:]:
                        emit_st_unit(hp + 1, pts_next, c, j)
                    emit_v_norm(hp, pyd[1], *v_order[1])
                    pyd.append(emit_v_matmul(hp, pts_cur, *v_order[3]))
                    emit_v_norm(hp, pyd[2], *v_order[2])
                    emit_v_norm(hp, pyd[3], *v_order[3])
                else:
                    pyd = [emit_v_matmul(hp, pts_cur, *v_order[0])]
                    for g in range(1, 4):
                        pyd.append(emit_v_matmul(hp, pts_cur, *v_order[g]))
                        emit_v_norm(hp, pyd[g - 1], *v_order[g - 1])
                    emit_v_norm(hp, pyd[3], *v_order[3])
                    emit_out_proj(range(8))
                pts_cur = pts_next

    nc.compile()
    return nc


_CACHED_NC = None


def _get_program():
    global _CACHED_NC
    if _CACHED_NC is None:
        _CACHED_NC = _build_program()
    return _CACHED_NC


def _prepare_in_maps(x, W_qkv, b_qkv, W_proj):
    x = np.asarray(x, np.float32)
    W_qkv = np.asarray(W_qkv, np.float32)
    W_proj = np.asarray(W_proj, np.float32)
    mm_np = ml_dtypes.bfloat16

    in_maps = []
    for core in range(NCORES):
        b, hg = core // 2, core % 2
        lo = hg * DL
        # [C, DL] -> [4dt|8cc, 128p, ...] pretiled so SBUF tiles are contiguous
        wq_s = W_qkv[:, lo : lo + DL].astype(mm_np)
        wk_s = W_qkv[:, C + lo : C + lo + DL].astype(mm_np)
        wv_s = W_qkv[:, 2 * C + lo : 2 * C + lo + DL].astype(mm_np)
        # wv: [1024, 512] -> [8cc, 128p, 512] -> [128p, 8cc, 512]
        wv_t = np.ascontiguousarray(wv_s.reshape(8, P, DL).transpose(1, 0, 2))
        # wq/wk: [1024, 512] -> [8cc, 128p, 4dt, 128n] -> [4dt, 128p, 8cc, 128n]
        wq_t = np.ascontiguousarray(
            wq_s.reshape(8, P, 4, P).transpose(2, 1, 0, 3)
        )
        wk_t = np.ascontiguousarray(
            wk_s.reshape(8, P, 4, P).transpose(2, 1, 0, 3)
        )
        # wp: [512, 1024] -> [4dc, 128p, 1024] -> [128p, 4dc, 1024]
        wp_s = W_proj[lo : lo + DL, :].astype(mm_np)
        wp_t = np.ascontiguousarray(wp_s.reshape(4, P, C).transpose(1, 0, 2))
        in_maps.append(
            {
                "xT": np.ascontiguousarray(x[b].T).astype(mm_np),
                "wq": wq_t,
                "wk": wk_t,
                "wv": wv_t,
                "wp": wp_t,
            }
        )
    return in_maps


def _reference_fallback(x, W_qkv, b_qkv, W_proj, b_proj):
    """Numpy reference path; only taken if b_qkv is unexpectedly nonzero."""
    x = np.asarray(x, np.float32)
    qkv = x @ np.asarray(W_qkv, np.float32) + np.asarray(b_qkv, np.float32)
    q, k, v = np.split(qkv, 3, axis=-1)

    def heads(t):
        return t.reshape(B, T, H, HD).transpose(0, 2, 1, 3)

    q, k, v = heads(q), heads(k), heads(v)
    s = np.einsum("bhqd,bhkd->bhqk", q, k) / np.sqrt(np.float32(HD))
    s = np.where(np.tril(np.ones((T, T), bool)), s, -np.inf)
    s -= s.max(axis=-1, keepdims=True)
    p = np.exp(s)
    p /= p.sum(axis=-1, keepdims=True)
    y = np.einsum("bhqk,bhkd->bhqd", p, v)
    y = y.transpose(0, 2, 1, 3).reshape(B, T, C)
    return y @ np.asarray(W_proj, np.float32) + np.asarray(b_proj, np.float32)


def run(inputs, trace=False):
    nc = _get_program()
    in_maps = _prepare_in_maps(
        inputs["x"], inputs["W_qkv"], inputs["b_qkv"], inputs["W_proj"]
    )
    res = run_bass_kernel_spmd(nc, in_maps, core_ids=list(range(NCORES)), trace=trace)
    b_proj = np.asarray(inputs["b_proj"], np.float32)
    out = np.empty((B, T, C), np.float32)
    for b in range(B):
        out[b] = (
            res.results[2 * b]["outp"].astype(np.float32)
            + res.results[2 * b + 1]["outp"].astype(np.float32)
            + b_proj
        )
    return out, res


def kernel(**inputs):
    if np.any(np.asarray(inputs["b_qkv"], np.float32)):
        return _reference_fallback(
            inputs["x"],
            inputs["W_qkv"],
            inputs["b_qkv"],
            inputs["W_proj"],
            inputs["b_proj"],
        )
    out, _ = run(inputs, trace=False)
    return out


# revision 35
# speedup vs baseline: 1.0611x; 1.0611x over previous
"""Causal self-attention (B=4, T=1024, C=1024, H=16) on 8 trn2 NeuronCores.

Sharding: core i handles batch b = i // 2 and head-group hg = i % 2
(8 heads = 512 of the 1024 channel dims). Each core computes

    qkv       = x[b] @ W_qkv[:, local]           (bf16 matmuls)
    P^T       = exp((k_h^T q_h) / 8) (causal)    (unstable softmax, bf16 P)
    y'^T      = [v_h | 1]^T @ P^T                (bf16, gives y^T + row-sums D)
    y^T       = y'^T / D                         (recip + bcast + DVE mul)
    partial   = y^T.T @ W_proj[local, :]         (bf16, bf16 output)

Host sums the two head-group partials per batch and adds b_proj.
b_qkv is all-zeros by construction (spec fill: zeros) so the qkv bias
add is elided on-device (host fallback guards the general case).

Weights are host-pretiled so every SBUF weight tile is one contiguous
DMA, all issued at the top of the program; the first x/wv chunks go on
the second HWDGE queue (qAct) split fine so the v-projection starts as
early as possible.

Pipeline per head pair hp (sts for hp+1, V groups for hp):
  [qk proj (hp+1) | PV g0,g1 (hp) | 6 S^T units (hp+1) | PV g2 | 6 more
   S^T units | PV g3 + norm chains (hp)]
(st units spread across the PV phase so the ACT exp stream, the pacer
at ~12.5us/iter, drains smoothly instead of bursting), with the
out-projection after the last pair's groups. A PE warm-up (24 dummy
matmuls on a zeroed tile) bridges the input-DMA window so the first
real matmul starts on a ramped clock. Engine budget per
iteration (PE 14.5us): ACT = 24 exps + 1 k-copy; DVE = q/k copies, 16
tri-mask muls, y'/D copies out of PSUM, reciprocals, normalize muls;
GpSimd = 4 partition-broadcasts; SP queue = input DMAs + D-row remaps +
odd-half yT staging; qAct queue = output DMAs.

Softmax denominator: the V matmul's ones-column lands row sums on PSUM
partition 64; DVE copies y'+D to SBUF (frees the PSUM bank), a DMA
remaps the D row to partition 0 (custom DVE reciprocal is base-0 only),
gpsimd broadcasts 1/D across partitions, DVE multiplies. Norm chains
are phase-split (all matmul+recip before bcast+mul) so no chain op ever
queues in front of the S->exp->mask->PV critical path; even halves
write yT directly, odd halves stage at base 0 and DMA-remap to
partitions 64-127 (DVE operands must share a partition base).

PSUM: 5-bank main pool (qk/S/out-proj) + 3-bank py pool (PV groups);
out-proj tiles alternate pools so 4 T-tiles can accumulate in flight
while the last head pair's norm chains drain. yT is split per (head
pair, query half) into 8 tiles to decouple out-proj reads from late
writers.
"""

import numpy as np
from contextlib import ExitStack

import ml_dtypes

import concourse.bacc as bacc
import concourse.tile as tile
import concourse.mybir as mybir
from concourse.bass_utils import run_bass_kernel_spmd
from concourse.masks import make_upper_triangular

B, T, C, H, HD = 4, 1024, 1024, 16, 64
NCORES = 8
HPG = 8            # heads per core
DL = HPG * HD      # 512 local channel dims per core
P = 128

F32 = mybir.dt.float32
BF16 = mybir.dt.bfloat16
EXP = mybir.ActivationFunctionType.Exp

PV = BF16
MMDT = BF16


def _build_program():
    nc = bacc.Bacc("TRN2", target_bir_lowering=False)

    xT = nc.dram_tensor("xT", [C, T], MMDT, kind="ExternalInput").ap()
    # host-pretiled: [p, cc, n] so each SBUF tile is one contiguous DMA
    wv = nc.dram_tensor("wv", [P, 8, DL], MMDT, kind="ExternalInput").ap()
    wq = nc.dram_tensor("wq", [4, P, 8, P], MMDT, kind="ExternalInput").ap()
    wk = nc.dram_tensor("wk", [4, P, 8, P], MMDT, kind="ExternalInput").ap()
    wp = nc.dram_tensor("wp", [P, 4, C], PV, kind="ExternalInput").ap()
    outp = nc.dram_tensor("outp", [T, C], PV, kind="ExternalOutput").ap()

    with tile.TileContext(nc) as tc:
        with ExitStack() as ctx:
            consts = ctx.enter_context(tc.tile_pool(name="consts", bufs=1))
            xt_pool = ctx.enter_context(tc.tile_pool(name="xt", bufs=8))
            w_pool = ctx.enter_context(tc.tile_pool(name="w", bufs=1))
            qk_pool = ctx.enter_context(tc.tile_pool(name="qk", bufs=2))
            v_pool = ctx.enter_context(tc.tile_pool(name="v", bufs=8))
            pt_pool = ctx.enter_context(tc.tile_pool(name="pt", bufs=52))
            yt_pool = ctx.enter_context(tc.tile_pool(name="yt", bufs=8))
            d_pool = ctx.enter_context(tc.tile_pool(name="d", bufs=12))
            out_pool = ctx.enter_context(tc.tile_pool(name="out", bufs=8))
            ps = ctx.enter_context(tc.tile_pool(name="ps", bufs=5, space="PSUM"))
            ps_py = ctx.enter_context(tc.tile_pool(name="psy", bufs=3, space="PSUM"))

            # ---- tiles ----
            xt = [xt_pool.tile([P, T], MMDT, name="xt") for _ in range(8)]
            wv_sb = w_pool.tile([P, 8, DL], MMDT, name="wv")
            wq_sb = [w_pool.tile([P, 8, P], MMDT, name="wq") for _ in range(4)]
            wk_sb = [w_pool.tile([P, 8, P], MMDT, name="wk") for _ in range(4)]
            wp_sb = w_pool.tile([P, 4, C], PV, name="wp")

            # ---- all input DMAs, just-in-time order ----
            # first chunks split fine + on the second HWDGE queue so the
            # v-projection can start as early as possible
            nc.scalar.dma_start(wv_sb[:, 0, :], wv[:, 0, :])
            nc.scalar.dma_start(xt[0][:, 0:256], xT[0:P, 0:256])
            nc.scalar.dma_start(xt[0][:, 256:512], xT[0:P, 256:512])
            nc.scalar.dma_start(xt[0][:, 512:1024], xT[0:P, 512:1024])
            nc.scalar.dma_start(wv_sb[:, 1, :], wv[:, 1, :])
            nc.scalar.dma_start(xt[1][:], xT[P : 2 * P, :])
            for cc in range(2, 8):
                nc.sync.dma_start(xt[cc][:], xT[P * cc : P * (cc + 1), :])
                nc.sync.dma_start(wv_sb[:, cc, :], wv[:, cc, :])
            nc.sync.dma_start(wq_sb[0][:], wq[0])
            nc.sync.dma_start(wk_sb[0][:], wk[0])
            nc.sync.dma_start(wp_sb[:], wp)
            for dt_ in range(1, 4):
                nc.sync.dma_start(wq_sb[dt_][:], wq[dt_])
                nc.sync.dma_start(wk_sb[dt_][:], wk[dt_])

            # ---- constants ----
            tri = consts.tile([P, P], PV, name="tri")  # 1 where tq >= s
            make_upper_triangular(nc, tri[:], val=1.0, diag=True)

            # PE warm-up: dummy matmuls on a zeroed tile while the first x
            # chunks stream in, so the PE clock is ramped to max before the
            # real v-projection starts (cold PE runs 2-4x slower for ~3us)
            warm = consts.tile([P, 512], PV, name="warm")
            nc.vector.memset(warm[:], 0.0)
            wps = ps.tile([P, 512], F32, name="ps")
            for _ in range(24):
                nc.tensor.matmul(
                    wps[:, 0:256],
                    lhsT=warm[:, 0:P],
                    rhs=warm[:, 0:256],
                    start=True,
                    stop=True,
                )

            # v tiles: [s=128, 8 heads x (64 dims + ones col)]
            v_sb = []
            for j in range(8):
                vt = v_pool.tile([P, HPG * (HD + 1)], PV, name="v")
                ones_cols = vt[:].rearrange("p (h e) -> p h e", e=HD + 1)[
                    :, :, HD : HD + 1
                ]
                nc.vector.memset(ones_cols, 1.0)
                v_sb.append(vt)

            qT_sb = [qk_pool.tile([P, 4, T], MMDT, name="qT") for _ in range(2)]
            kT_sb = [qk_pool.tile([P, 4, T], MMDT, name="kT") for _ in range(2)]
            # split per (head pair, query half) so late writers never gate
            # early out-proj reads through coarse dependency tracking
            yT_sb = [
                [yt_pool.tile([P, 512], PV, name="yT") for _ in range(2)]
                for _ in range(4)
            ]

            # ---- v projection (needed by every head pair), 2 waves ----
            for wave in range(2):
                ps_t = [ps.tile([P, 512], F32, name="ps") for _ in range(4)]
                for cc in range(8):
                    for wt_ in range(4):
                        tt = 4 * wave + wt_
                        nc.tensor.matmul(
                            ps_t[wt_][:, :],
                            lhsT=xt[cc][:, P * tt : P * (tt + 1)],
                            rhs=wv_sb[:, cc, :],
                            start=(cc == 0),
                            stop=(cc == 7),
                        )
                for wt_ in range(4):
                    tt = 4 * wave + wt_
                    out_ap = v_sb[tt][:].rearrange("p (h e) -> p h e", e=HD + 1)[
                        :, :, 0:HD
                    ]
                    in_ap = ps_t[wt_][:].rearrange("p (h e) -> p h e", e=HD)
                    nc.scalar.copy(out_ap, in_ap)

            def issue_qk(dt_):
                """q and k projections for head pair dt_ (128 channel dims)."""
                buf = dt_ % 2
                for w_sb, dest, cengs in (
                    (wq_sb[dt_], qT_sb[buf],
                     (nc.vector.tensor_copy, nc.vector.tensor_copy)),
                    (wk_sb[dt_], kT_sb[buf],
                     (nc.vector.tensor_copy, nc.scalar.copy)),
                ):
                    pst = [ps.tile([P, 512], F32, name="ps") for _ in range(2)]
                    for cc in range(8):
                        for tch in range(2):
                            nc.tensor.matmul(
                                pst[tch][:, :],
                                lhsT=w_sb[:, cc, :],
                                rhs=xt[cc][:, 512 * tch : 512 * (tch + 1)],
                                start=(cc == 0),
                                stop=(cc == 7),
                            )
                    for tch in range(2):
                        cengs[tch](
                            dest[:, dt_, 512 * tch : 512 * (tch + 1)], pst[tch][:, :]
                        )

            def emit_st_unit(hp, pts, c, j):
                """One S^T block (both halves) + exp + causal mask."""
                buf = hp % 2
                off = max(0, P * (j - 4 * c))
                n = 512 - off
                for half in range(2):
                    pr = 64 * half
                    pss = ps.tile([P, 512], F32, name="ps")
                    nc.tensor.matmul(
                        pss[:, :n],
                        lhsT=kT_sb[buf][pr : pr + 64, hp, P * j : P * (j + 1)],
                        rhs=qT_sb[buf][pr : pr + 64, hp, 512 * c + off : 512 * (c + 1)],
                        start=True,
                        stop=True,
                    )
                    pt = pt_pool.tile([P, 512], PV, name="pt")
                    nc.scalar.activation(
                        out=pt[:, off:512], in_=pss[:, :n], func=EXP, scale=0.125
                    )
                    if j >= 4 * c:
                        nc.vector.tensor_mul(
                            pt[:, off : off + P], pt[:, off : off + P], tri[:]
                        )
                    pts[(half, c, j)] = pt

            def emit_v_matmul(hp, pts, half, c):
                """V matmuls + D-row extract + reciprocal for one (half, c).

                DVE copies y'+D out of PSUM (freeing the bank), a sync-queue
                DMA remaps the D row to partition 0, DVE reciprocals it there
                (custom op, base-0 only). The bcast+mul live in emit_v_norm so
                recips of later groups never queue behind muls of earlier ones
                on DVE.
                """
                h = 2 * hp + half
                jmax = 4 * c + 3
                py = ps_py.tile([P, 512], F32, name="py")
                for j in range(jmax + 1):
                    off = max(0, P * (j - 4 * c))
                    nc.tensor.matmul(
                        py[0 : HD + 1, off:512],
                        lhsT=v_sb[j][:, (HD + 1) * h : (HD + 1) * (h + 1)],
                        rhs=pts[(half, c, j)][:, off:512],
                        start=(j == 0),
                        stop=(j == jmax),
                    )
                yc = d_pool.tile([HD + 1, 512], F32, name="yc")
                nc.vector.tensor_copy(yc[:], py[0 : HD + 1, :])
                d2 = d_pool.tile([P, 512], F32, name="d")
                nc.sync.dma_start(d2[0:1, :], yc[HD : HD + 1, :])
                # custom DVE op only at partition base 0 (HW quirk)
                nc.vector.reciprocal_approx_fast(d2[0:1, :], d2[0:1, :])
                return yc, d2

            def emit_v_norm(hp, pyd2, half, c):
                """Broadcast 1/D and multiply into yT (odd halves staged)."""
                yc, d2 = pyd2
                nc.gpsimd.partition_broadcast(d2[0:HD, :], d2[0:1, :])
                pr = 64 * half
                dst = yT_sb[hp][c][pr : pr + HD, :]
                if half == 0:
                    nc.vector.tensor_mul(dst, yc[0:HD, :], d2[0:HD, :])
                else:
                    stg = d_pool.tile([HD, 512], PV, name="stg")
                    nc.vector.tensor_mul(stg[:], yc[0:HD, :], d2[0:HD, :])
                    nc.sync.dma_start(dst, stg[:])

            def emit_out_proj(tts):
                for tt in tts:
                    pool, pnm = (ps, "ps") if tt % 2 == 0 else (ps_py, "py")
                    pouts = [pool.tile([P, 512], F32, name=pnm) for _ in range(2)]
                    for dc in range(4):
                        for cch in range(2):
                            nc.tensor.matmul(
                                pouts[cch][:, :],
                                lhsT=yT_sb[dc][tt // 4][
                                    :, P * (tt % 4) : P * (tt % 4 + 1)
                                ],
                                rhs=wp_sb[:, dc, 512 * cch : 512 * (cch + 1)],
                                start=(dc == 0),
                                stop=(dc == 3),
                            )
                    for cch in range(2):
                        ot = out_pool.tile([P, 512], PV, name="out")
                        if cch == 0:
                            nc.scalar.copy(ot[:], pouts[cch][:])
                        else:
                            nc.vector.tensor_copy(ot[:], pouts[cch][:])
                        nc.scalar.dma_start(
                            outp[P * tt : P * (tt + 1), 512 * cch : 512 * (cch + 1)],
                            ot[:],
                        )

            # ---- pipelined qk + attention ----
            # Per head pair: qk proj for the next pair, ALL 12 S^T units for
            # the next pair, then this pair's 4 V groups (c=0 halves first).
            # st units precede v groups on every engine stream, so the
            # normalization chain never blocks the S->PV critical path.
            st_order = [(c, j) for c in range(2) for j in range(4 * c + 4)]
            v_order = [(0, 0), (1, 0), (0, 1), (1, 1)]  # c=0 halves first
            issue_qk(0)
            pts_cur = {}
            for c, j in st_order:
                emit_st_unit(0, pts_cur, c, j)
            for hp in range(4):
                pts_next = {}
                if hp + 1 < 4:
                    issue_qk(hp + 1)
                    pyd = [emit_v_matmul(hp, pts_cur, *v_order[0])]
                    pyd.append(emit_v_matmul(hp, pts_cur, *v_order[1]))
                    for c, j in st_order[:6]:
                        emit_st_unit(hp + 1, pts_next, c, j)
                    emit_v_norm(hp, pyd[0], *v_order[0])
                    pyd.append(emit_v_matmul(hp, pts_cur, *v_order[2]))
                    for c, j in st_order[6:]:
                        emit_st_unit(hp + 1, pts_next, c, j)
                    emit_v_norm(hp, pyd[1], *v_order[1])
                    pyd.append(emit_v_matmul(hp, pts_cur, *v_order[3]))
                    emit_v_norm(hp, pyd[2], *v_order[2])
                    emit_v_norm(hp, pyd[3], *v_order[3])
                else:
                    pyd = [emit_v_matmul(hp, pts_cur, *v_order[0])]
                    for g in range(1, 4):
                        pyd.append(emit_v_matmul(hp, pts_cur, *v_order[g]))
                        emit_v_norm(hp, pyd[g - 1], *v_order[g - 1])
                    emit_v_norm(hp, pyd[3], *v_order[3])
                    emit_out_proj(range(8))
                pts_cur = pts_next

    nc.compile()
    return nc


_CACHED_NC = None


def _get_program():
    global _CACHED_NC
    if _CACHED_NC is None:
        _CACHED_NC = _build_program()
    return _CACHED_NC


def _prepare_in_maps(x, W_qkv, b_qkv, W_proj):
    x = np.asarray(x, np.float32)
    W_qkv = np.asarray(W_qkv, np.float32)
    W_proj = np.asarray(W_proj, np.float32)
    mm_np = ml_dtypes.bfloat16

    in_maps = []
    for core in range(NCORES):
        b, hg = core // 2, core % 2
        lo = hg * DL
        # [C, DL] -> [4dt|8cc, 128p, ...] pretiled so SBUF tiles are contiguous
        wq_s = W_qkv[:, lo : lo + DL].astype(mm_np)
        wk_s = W_qkv[:, C + lo : C + lo + DL].astype(mm_np)
        wv_s = W_qkv[:, 2 * C + lo : 2 * C + lo + DL].astype(mm_np)
        # wv: [1024, 512] -> [8cc, 128p, 512] -> [128p, 8cc, 512]
        wv_t = np.ascontiguousarray(wv_s.reshape(8, P, DL).transpose(1, 0, 2))
        # wq/wk: [1024, 512] -> [8cc, 128p, 4dt, 128n] -> [4dt, 128p, 8cc, 128n]
        wq_t = np.ascontiguousarray(
            wq_s.reshape(8, P, 4, P).transpose(2, 1, 0, 3)
        )
        wk_t = np.ascontiguousarray(
            wk_s.reshape(8, P, 4, P).transpose(2, 1, 0, 3)
        )
        # wp: [512, 1024] -> [4dc, 128p, 1024] -> [128p, 4dc, 1024]
        wp_s = W_proj[lo : lo + DL, :].astype(mm_np)
        wp_t = np.ascontiguousarray(wp_s.reshape(4, P, C).transpose(1, 0, 2))
        in_maps.append(
            {
                "xT": np.ascontiguousarray(x[b].T).astype(mm_np),
                "wq": wq_t,
                "wk": wk_t,
                "wv": wv_t,
                "wp": wp_t,
            }
        )
    return in_maps


def _reference_fallback(x, W_qkv, b_qkv, W_proj, b_proj):
    """Numpy reference path; only taken if b_qkv is unexpectedly nonzero."""
    x = np.asarray(x, np.float32)
    qkv = x @ np.asarray(W_qkv, np.float32) + np.asarray(b_qkv, np.float32)
    q, k, v = np.split(qkv, 3, axis=-1)

    def heads(t):
        return t.reshape(B, T, H, HD).transpose(0, 2, 1, 3)

    q, k, v = heads(q), heads(k), heads(v)
    s = np.einsum("bhqd,bhkd->bhqk", q, k) / np.sqrt(np.float32(HD))
    s = np.where(np.tril(np.ones((T, T), bool)), s, -np.inf)
    s -= s.max(axis=-1, keepdims=True)
    p = np.exp(s)
    p /= p.sum(axis=-1, keepdims=True)
    y = np.einsum("bhqk,bhkd->bhqd", p, v)
    y = y.transpose(0, 2, 1, 3).reshape(B, T, C)
    return y @ np.asarray(W_proj, np.float32) + np.asarray(b_proj, np.float32)


def run(inputs, trace=False):
    nc = _get_program()
    in_maps = _prepare_in_maps(
        inputs["x"], inputs["W_qkv"], inputs["b_qkv"], inputs["W_proj"]
    )
    res = run_bass_kernel_spmd(nc, in_maps, core_ids=list(range(NCORES)), trace=trace)
    b_proj = np.asarray(inputs["b_proj"], np.float32)
    out = np.empty((B, T, C), np.float32)
    for b in range(B):
        out[b] = (
            res.results[2 * b]["outp"].astype(np.float32)
            + res.results[2 * b + 1]["outp"].astype(np.float32)
            + b_proj
        )
    return out, res


def kernel(**inputs):
    if np.any(np.asarray(inputs["b_qkv"], np.float32)):
        return _reference_fallback(
            inputs["x"],
            inputs["W_qkv"],
            inputs["b_qkv"],
            inputs["W_proj"],
            inputs["b_proj"],
        )
    out, _ = run(inputs, trace=False)
    return out


# revision 36
# speedup vs baseline: 1.0789x; 1.0168x over previous
"""Causal self-attention (B=4, T=1024, C=1024, H=16) on 8 trn2 NeuronCores.

Sharding: core i handles batch b = i // 2 and head-group hg = i % 2
(8 heads = 512 of the 1024 channel dims). Each core computes

    qkv       = x[b] @ W_qkv[:, local]           (bf16 matmuls)
    P^T       = exp((k_h^T q_h) / 8) (causal)    (unstable softmax, bf16 P)
    y'^T      = [v_h | 1]^T @ P^T                (bf16, gives y^T + row-sums D)
    y^T       = y'^T / D                         (recip + bcast + DVE mul)
    partial   = y^T.T @ W_proj[local, :]         (bf16, bf16 output)

Host sums the two head-group partials per batch and adds b_proj.
b_qkv is all-zeros by construction (spec fill: zeros) so the qkv bias
add is elided on-device (host fallback guards the general case).

Weights are host-pretiled so every SBUF weight tile is one contiguous
DMA, all issued at the top of the program; the first x/wv chunks go on
the second HWDGE queue (qAct) split fine so the v-projection starts as
early as possible.

Pipeline per head pair hp (sts for hp+1, V groups for hp):
  [qk proj (hp+1) | PV g0,g1 (hp) | 6 S^T units (hp+1) | PV g2 | 6 more
   S^T units | PV g3 + norm chains (hp)]
(st units spread across the PV phase so the ACT exp stream, the pacer
at ~12.5us/iter, drains smoothly instead of bursting), with the
out-projection after the last pair's groups. A PE warm-up (24 dummy
matmuls on a zeroed tile) bridges the input-DMA window so the first
real matmul starts on a ramped clock. Engine budget per
iteration (PE 14.5us): ACT = 24 exps + 1 k-copy; DVE = q/k copies, 16
tri-mask muls, y'/D copies out of PSUM, reciprocals, normalize muls;
GpSimd = 4 partition-broadcasts; SP queue = input DMAs + D-row remaps +
odd-half yT staging; qAct queue = output DMAs.

Softmax denominator: the V matmul's ones-column lands row sums on PSUM
partition 64; DVE copies y'+D to SBUF (frees the PSUM bank), a DMA
remaps the D row to partition 0 (custom DVE reciprocal is base-0 only),
gpsimd broadcasts 1/D across partitions, DVE multiplies. Norm chains
are phase-split (all matmul+recip before bcast+mul) so no chain op ever
queues in front of the S->exp->mask->PV critical path; even halves
write yT directly, odd halves stage at base 0 and DMA-remap to
partitions 64-127 (DVE operands must share a partition base).

PSUM: 5-bank main pool (qk/S/out-proj) + 3-bank py pool (PV groups);
out-proj tiles alternate pools so 4 T-tiles can accumulate in flight
while the last head pair's norm chains drain. yT is split per (head
pair, query half) into 8 tiles to decouple out-proj reads from late
writers.
"""

import numpy as np
from contextlib import ExitStack

import ml_dtypes

import concourse.bacc as bacc
import concourse.tile as tile
import concourse.mybir as mybir
from concourse.bass_utils import run_bass_kernel_spmd
from concourse.masks import make_upper_triangular

B, T, C, H, HD = 4, 1024, 1024, 16, 64
NCORES = 8
HPG = 8            # heads per core
DL = HPG * HD      # 512 local channel dims per core
P = 128

F32 = mybir.dt.float32
BF16 = mybir.dt.bfloat16
EXP = mybir.ActivationFunctionType.Exp

PV = BF16
MMDT = BF16


def _build_program():
    nc = bacc.Bacc("TRN2", target_bir_lowering=False)

    xT = nc.dram_tensor("xT", [C, T], MMDT, kind="ExternalInput").ap()
    # host-pretiled: [p, cc, n] so each SBUF tile is one contiguous DMA
    wv = nc.dram_tensor("wv", [P, 8, DL], MMDT, kind="ExternalInput").ap()
    wq = nc.dram_tensor("wq", [4, P, 8, P], MMDT, kind="ExternalInput").ap()
    wk = nc.dram_tensor("wk", [4, P, 8, P], MMDT, kind="ExternalInput").ap()
    wp = nc.dram_tensor("wp", [P, 4, C], PV, kind="ExternalInput").ap()
    outp = nc.dram_tensor("outp", [T, C], PV, kind="ExternalOutput").ap()

    with tile.TileContext(nc) as tc:
        with ExitStack() as ctx:
            consts = ctx.enter_context(tc.tile_pool(name="consts", bufs=1))
            xt_pool = ctx.enter_context(tc.tile_pool(name="xt", bufs=8))
            w_pool = ctx.enter_context(tc.tile_pool(name="w", bufs=1))
            qk_pool = ctx.enter_context(tc.tile_pool(name="qk", bufs=2))
            v_pool = ctx.enter_context(tc.tile_pool(name="v", bufs=8))
            pt_pool = ctx.enter_context(tc.tile_pool(name="pt", bufs=52))
            yt_pool = ctx.enter_context(tc.tile_pool(name="yt", bufs=8))
            d_pool = ctx.enter_context(tc.tile_pool(name="d", bufs=12))
            out_pool = ctx.enter_context(tc.tile_pool(name="out", bufs=8))
            ps = ctx.enter_context(tc.tile_pool(name="ps", bufs=5, space="PSUM"))
            ps_py = ctx.enter_context(tc.tile_pool(name="psy", bufs=3, space="PSUM"))

            # ---- tiles ----
            xt = [xt_pool.tile([P, T], MMDT, name="xt") for _ in range(8)]
            wv_sb = w_pool.tile([P, 8, DL], MMDT, name="wv")
            wq_sb = [w_pool.tile([P, 8, P], MMDT, name="wq") for _ in range(4)]
            wk_sb = [w_pool.tile([P, 8, P], MMDT, name="wk") for _ in range(4)]
            wp_sb = w_pool.tile([P, 4, C], PV, name="wp")

            # ---- all input DMAs, just-in-time order ----
            # first chunks split fine + on the second HWDGE queue so the
            # v-projection can start as early as possible
            nc.scalar.dma_start(wv_sb[:, 0, :], wv[:, 0, :])
            nc.scalar.dma_start(xt[0][:, 0:256], xT[0:P, 0:256])
            nc.scalar.dma_start(xt[0][:, 256:512], xT[0:P, 256:512])
            nc.scalar.dma_start(xt[0][:, 512:1024], xT[0:P, 512:1024])
            nc.scalar.dma_start(wv_sb[:, 1, :], wv[:, 1, :])
            nc.scalar.dma_start(xt[1][:], xT[P : 2 * P, :])
            for cc in range(2, 8):
                nc.sync.dma_start(xt[cc][:], xT[P * cc : P * (cc + 1), :])
                nc.sync.dma_start(wv_sb[:, cc, :], wv[:, cc, :])
            nc.sync.dma_start(wq_sb[0][:], wq[0])
            nc.sync.dma_start(wk_sb[0][:], wk[0])
            nc.sync.dma_start(wp_sb[:], wp)
            for dt_ in range(1, 4):
                nc.sync.dma_start(wq_sb[dt_][:], wq[dt_])
                nc.sync.dma_start(wk_sb[dt_][:], wk[dt_])

            # ---- constants ----
            tri = consts.tile([P, P], PV, name="tri")  # 1 where tq >= s
            make_upper_triangular(nc, tri[:], val=1.0, diag=True)

            # PE warm-up: dummy matmuls on a zeroed tile while the first x
            # chunks stream in, so the PE clock is ramped to max before the
            # real v-projection starts (cold PE runs 2-4x slower for ~3us)
            warm = consts.tile([P, 512], PV, name="warm")
            nc.gpsimd.memset(warm[:], 0.0)
            wps = ps.tile([P, 512], F32, name="ps")
            for _ in range(24):
                nc.tensor.matmul(
                    wps[:, 0:256],
                    lhsT=warm[:, 0:P],
                    rhs=warm[:, 0:256],
                    start=True,
                    stop=True,
                )

            # v tiles: [s=128, 8 heads x (64 dims + ones col)]
            v_sb = []
            for j in range(8):
                vt = v_pool.tile([P, HPG * (HD + 1)], PV, name="v")
                ones_cols = vt[:].rearrange("p (h e) -> p h e", e=HD + 1)[
                    :, :, HD : HD + 1
                ]
                nc.vector.memset(ones_cols, 1.0)
                v_sb.append(vt)

            qT_sb = [qk_pool.tile([P, 4, T], MMDT, name="qT") for _ in range(2)]
            kT_sb = [qk_pool.tile([P, 4, T], MMDT, name="kT") for _ in range(2)]
            # split per (head pair, query half) so late writers never gate
            # early out-proj reads through coarse dependency tracking
            yT_sb = [
                [yt_pool.tile([P, 512], PV, name="yT") for _ in range(2)]
                for _ in range(4)
            ]

            # ---- v projection (needed by every head pair), 2 waves ----
            for wave in range(2):
                ps_t = [ps.tile([P, 512], F32, name="ps") for _ in range(4)]
                for cc in range(8):
                    for wt_ in range(4):
                        tt = 4 * wave + wt_
                        nc.tensor.matmul(
                            ps_t[wt_][:, :],
                            lhsT=xt[cc][:, P * tt : P * (tt + 1)],
                            rhs=wv_sb[:, cc, :],
                            start=(cc == 0),
                            stop=(cc == 7),
                        )
                for wt_ in range(4):
                    tt = 4 * wave + wt_
                    out_ap = v_sb[tt][:].rearrange("p (h e) -> p h e", e=HD + 1)[
                        :, :, 0:HD
                    ]
                    in_ap = ps_t[wt_][:].rearrange("p (h e) -> p h e", e=HD)
                    nc.scalar.copy(out_ap, in_ap)

            def issue_qk(dt_):
                """q and k projections for head pair dt_ (128 channel dims)."""
                buf = dt_ % 2
                for w_sb, dest, cengs in (
                    (wq_sb[dt_], qT_sb[buf],
                     (nc.vector.tensor_copy, nc.vector.tensor_copy)),
                    (wk_sb[dt_], kT_sb[buf],
                     (nc.vector.tensor_copy, nc.scalar.copy)),
                ):
                    pst = [ps.tile([P, 512], F32, name="ps") for _ in range(2)]
                    for cc in range(8):
                        for tch in range(2):
                            nc.tensor.matmul(
                                pst[tch][:, :],
                                lhsT=w_sb[:, cc, :],
                                rhs=xt[cc][:, 512 * tch : 512 * (tch + 1)],
                                start=(cc == 0),
                                stop=(cc == 7),
                            )
                    for tch in range(2):
                        cengs[tch](
                            dest[:, dt_, 512 * tch : 512 * (tch + 1)], pst[tch][:, :]
                        )

            def emit_st_unit(hp, pts, c, j):
                """One S^T block (both halves) + exp + causal mask."""
                buf = hp % 2
                off = max(0, P * (j - 4 * c))
                n = 512 - off
                for half in range(2):
                    pr = 64 * half
                    pss = ps.tile([P, 512], F32, name="ps")
                    nc.tensor.matmul(
                        pss[:, :n],
                        lhsT=kT_sb[buf][pr : pr + 64, hp, P * j : P * (j + 1)],
                        rhs=qT_sb[buf][pr : pr + 64, hp, 512 * c + off : 512 * (c + 1)],
                        start=True,
                        stop=True,
                    )
                    pt = pt_pool.tile([P, 512], PV, name="pt")
                    nc.scalar.activation(
                        out=pt[:, off:512], in_=pss[:, :n], func=EXP, scale=0.125
                    )
                    if j >= 4 * c:
                        nc.vector.tensor_mul(
                            pt[:, off : off + P], pt[:, off : off + P], tri[:]
                        )
                    pts[(half, c, j)] = pt

            def emit_v_matmul(hp, pts, half, c):
                """V matmuls + D-row extract + reciprocal for one (half, c).

                DVE copies y'+D out of PSUM (freeing the bank), a sync-queue
                DMA remaps the D row to partition 0, DVE reciprocals it there
                (custom op, base-0 only). The bcast+mul live in emit_v_norm so
                recips of later groups never queue behind muls of earlier ones
                on DVE.
                """
                h = 2 * hp + half
                jmax = 4 * c + 3
                py = ps_py.tile([P, 512], F32, name="py")
                for j in range(jmax + 1):
                    off = max(0, P * (j - 4 * c))
                    nc.tensor.matmul(
                        py[0 : HD + 1, off:512],
                        lhsT=v_sb[j][:, (HD + 1) * h : (HD + 1) * (h + 1)],
                        rhs=pts[(half, c, j)][:, off:512],
                        start=(j == 0),
                        stop=(j == jmax),
                    )
                yc = d_pool.tile([HD + 1, 512], F32, name="yc")
                nc.vector.tensor_copy(yc[:], py[0 : HD + 1, :])
                d2 = d_pool.tile([P, 512], F32, name="d")
                nc.sync.dma_start(d2[0:1, :], yc[HD : HD + 1, :])
                # custom DVE op only at partition base 0 (HW quirk)
                nc.vector.reciprocal_approx_fast(d2[0:1, :], d2[0:1, :])
                return yc, d2

            def emit_v_norm(hp, pyd2, half, c):
                """Broadcast 1/D and multiply into yT (odd halves staged)."""
                yc, d2 = pyd2
                nc.gpsimd.partition_broadcast(d2[0:HD, :], d2[0:1, :])
                pr = 64 * half
                dst = yT_sb[hp][c][pr : pr + HD, :]
                if half == 0:
                    nc.vector.tensor_mul(dst, yc[0:HD, :], d2[0:HD, :])
                else:
                    stg = d_pool.tile([HD, 512], PV, name="stg")
                    nc.vector.tensor_mul(stg[:], yc[0:HD, :], d2[0:HD, :])
                    nc.sync.dma_start(dst, stg[:])

            def emit_out_proj(tts):
                for tt in tts:
                    pool, pnm = (ps, "ps") if tt % 2 == 0 else (ps_py, "py")
                    pouts = [pool.tile([P, 512], F32, name=pnm) for _ in range(2)]
                    for dc in range(4):
                        for cch in range(2):
                            nc.tensor.matmul(
                                pouts[cch][:, :],
                                lhsT=yT_sb[dc][tt // 4][
                                    :, P * (tt % 4) : P * (tt % 4 + 1)
                                ],
                                rhs=wp_sb[:, dc, 512 * cch : 512 * (cch + 1)],
                                start=(dc == 0),
                                stop=(dc == 3),
                            )
                    for cch in range(2):
                        ot = out_pool.tile([P, 512], PV, name="out")
                        if cch == 0:
                            nc.scalar.copy(ot[:], pouts[cch][:])
                        else:
                            nc.vector.tensor_copy(ot[:], pouts[cch][:])
                        nc.scalar.dma_start(
                            outp[P * tt : P * (tt + 1), 512 * cch : 512 * (cch + 1)],
                            ot[:],
                        )

            # ---- pipelined qk + attention ----
            # Per head pair: qk proj for the next pair, ALL 12 S^T units for
            # the next pair, then this pair's 4 V groups (c=0 halves first).
            # st units precede v groups on every engine stream, so the
            # normalization chain never blocks the S->PV critical path.
            st_order = [(c, j) for c in range(2) for j in range(4 * c + 4)]
            v_order = [(0, 0), (1, 0), (0, 1), (1, 1)]  # c=0 halves first
            issue_qk(0)
            pts_cur = {}
            for c, j in st_order:
                emit_st_unit(0, pts_cur, c, j)
            for hp in range(4):
                pts_next = {}
                if hp + 1 < 4:
                    issue_qk(hp + 1)
                    pyd = [emit_v_matmul(hp, pts_cur, *v_order[0])]
                    pyd.append(emit_v_matmul(hp, pts_cur, *v_order[1]))
                    for c, j in st_order[:6]:
                        emit_st_unit(hp + 1, pts_next, c, j)
                    emit_v_norm(hp, pyd[0], *v_order[0])
                    pyd.append(emit_v_matmul(hp, pts_cur, *v_order[2]))
                    for c, j in st_order[6:]:
                        emit_st_unit(hp + 1, pts_next, c, j)
                    emit_v_norm(hp, pyd[1], *v_order[1])
                    pyd.append(emit_v_matmul(hp, pts_cur, *v_order[3]))
                    emit_v_norm(hp, pyd[2], *v_order[2])
                    emit_v_norm(hp, pyd[3], *v_order[3])
                else:
                    pyd = [emit_v_matmul(hp, pts_cur, *v_order[0])]
                    for g in range(1, 4):
                        pyd.append(emit_v_matmul(hp, pts_cur, *v_order[g]))
                        emit_v_norm(hp, pyd[g - 1], *v_order[g - 1])
                    emit_v_norm(hp, pyd[3], *v_order[3])
                    emit_out_proj(range(8))
                pts_cur = pts_next

    nc.compile()
    return nc


_CACHED_NC = None


def _get_program():
    global _CACHED_NC
    if _CACHED_NC is None:
        _CACHED_NC = _build_program()
    return _CACHED_NC


def _prepare_in_maps(x, W_qkv, b_qkv, W_proj):
    x = np.asarray(x, np.float32)
    W_qkv = np.asarray(W_qkv, np.float32)
    W_proj = np.asarray(W_proj, np.float32)
    mm_np = ml_dtypes.bfloat16

    in_maps = []
    for core in range(NCORES):
        b, hg = core // 2, core % 2
        lo = hg * DL
        # [C, DL] -> [4dt|8cc, 128p, ...] pretiled so SBUF tiles are contiguous
        wq_s = W_qkv[:, lo : lo + DL].astype(mm_np)
        wk_s = W_qkv[:, C + lo : C + lo + DL].astype(mm_np)
        wv_s = W_qkv[:, 2 * C + lo : 2 * C + lo + DL].astype(mm_np)
        # wv: [1024, 512] -> [8cc, 128p, 512] -> [128p, 8cc, 512]
        wv_t = np.ascontiguousarray(wv_s.reshape(8, P, DL).transpose(1, 0, 2))
        # wq/wk: [1024, 512] -> [8cc, 128p, 4dt, 128n] -> [4dt, 128p, 8cc, 128n]
        wq_t = np.ascontiguousarray(
            wq_s.reshape(8, P, 4, P).transpose(2, 1, 0, 3)
        )
        wk_t = np.ascontiguousarray(
            wk_s.reshape(8, P, 4, P).transpose(2, 1, 0, 3)
        )
        # wp: [512, 1024] -> [4dc, 128p, 1024] -> [128p, 4dc, 1024]
        wp_s = W_proj[lo : lo + DL, :].astype(mm_np)
        wp_t = np.ascontiguousarray(wp_s.reshape(4, P, C).transpose(1, 0, 2))
        in_maps.append(
            {
                "xT": np.ascontiguousarray(x[b].T).astype(mm_np),
                "wq": wq_t,
                "wk": wk_t,
                "wv": wv_t,
                "wp": wp_t,
            }
        )
    return in_maps


def _reference_fallback(x, W_qkv, b_qkv, W_proj, b_proj):
    """Numpy reference path; only taken if b_qkv is unexpectedly nonzero."""
    x = np.asarray(x, np.float32)
    qkv = x @ np.asarray(W_qkv, np.float32) + np.asarray(b_qkv, np.float32)
    q, k, v = np.split(qkv, 3, axis=-1)

    def heads(t):
        return t.reshape(B, T, H, HD).transpose(0, 2, 1, 3)

    q, k, v = heads(q), heads(k), heads(v)
    s = np.einsum("bhqd,bhkd->bhqk", q, k) / np.sqrt(np.float32(HD))
    s = np.where(np.tril(np.ones((T, T), bool)), s, -np.inf)
    s -= s.max(axis=-1, keepdims=True)
    p = np.exp(s)
    p /= p.sum(axis=-1, keepdims=True)
    y = np.einsum("bhqk,bhkd->bhqd", p, v)
    y = y.transpose(0, 2, 1, 3).reshape(B, T, C)
    return y @ np.asarray(W_proj, np.float32) + np.asarray(b_proj, np.float32)


def run(inputs, trace=False):
    nc = _get_program()
    in_maps = _prepare_in_maps(
        inputs["x"], inputs["W_qkv"], inputs["b_qkv"], inputs["W_proj"]
    )
    res = run_bass_kernel_spmd(nc, in_maps, core_ids=list(range(NCORES)), trace=trace)
    b_proj = np.asarray(inputs["b_proj"], np.float32)
    out = np.empty((B, T, C), np.float32)
    for b in range(B):
        out[b] = (
            res.results[2 * b]["outp"].astype(np.float32)
            + res.results[2 * b + 1]["outp"].astype(np.float32)
            + b_proj
        )
    return out, res


def kernel(**inputs):
    if np.any(np.asarray(inputs["b_qkv"], np.float32)):
        return _reference_fallback(
            inputs["x"],
            inputs["W_qkv"],
            inputs["b_qkv"],
            inputs["W_proj"],
            inputs["b_proj"],
        )
    out, _ = run(inputs, trace=False)
    return out


# revision 39
# speedup vs baseline: 1.0870x; 1.0075x over previous
"""Causal self-attention (B=4, T=1024, C=1024, H=16) on 8 trn2 NeuronCores.

Sharding: core i handles batch b = i // 2 and head-group hg = i % 2
(8 heads = 512 of the 1024 channel dims). Each core computes

    qkv       = x[b] @ W_qkv[:, local]           (bf16 matmuls)
    P^T       = exp((k_h^T q_h) / 8) (causal)    (unstable softmax, bf16 P)
    y'^T      = [v_h | 1]^T @ P^T                (bf16, gives y^T + row-sums D)
    y^T       = y'^T / D                         (recip + bcast + DVE mul)
    partial   = y^T.T @ W_proj[local, :]         (bf16, bf16 output)

Host sums the two head-group partials per batch and adds b_proj.
b_qkv is all-zeros by construction (spec fill: zeros) so the qkv bias
add is elided on-device (host fallback guards the general case).

Weights are host-pretiled so every SBUF weight tile is one contiguous
DMA, all issued at the top of the program; the first x/wv chunks go on
the second HWDGE queue (qAct) split fine so the v-projection starts as
early as possible.

Pipeline per head pair hp (sts for hp+1, V groups for hp):
  [qk proj (hp+1) | PV g0,g1 (hp) | 12 S^T units (hp+1) | PV g2,g3 +
   norm chains (hp)]
with the out-projection after the last pair's groups. Engine budget per
iteration (PE 14.5us): ACT = 24 exps + 1 k-copy; DVE = q/k copies, 16
tri-mask muls, y'/D copies out of PSUM, reciprocals, normalize muls;
GpSimd = 4 partition-broadcasts; SP queue = input DMAs + D-row remaps +
odd-half yT staging; qAct queue = output DMAs.

Softmax denominator: the V matmul's ones-column lands row sums on PSUM
partition 64; DVE copies y'+D to SBUF (frees the PSUM bank), a DMA
remaps the D row to partition 0 (custom DVE reciprocal is base-0 only),
gpsimd broadcasts 1/D across partitions, DVE multiplies. Norm chains
are phase-split (all matmul+recip before bcast+mul) so no chain op ever
queues in front of the S->exp->mask->PV critical path; even halves
write yT directly, odd halves stage at base 0 and DMA-remap to
partitions 64-127 (DVE operands must share a partition base).

PSUM: 5-bank main pool (qk/S/out-proj) + 3-bank py pool (PV groups);
out-proj tiles alternate pools so 4 T-tiles can accumulate in flight
while the last head pair's norm chains drain. yT is split per (head
pair, query half) into 8 tiles to decouple out-proj reads from late
writers.
"""

import numpy as np
from contextlib import ExitStack

import ml_dtypes

import concourse.bacc as bacc
import concourse.tile as tile
import concourse.mybir as mybir
from concourse.bass_utils import run_bass_kernel_spmd
from concourse.masks import make_upper_triangular

B, T, C, H, HD = 4, 1024, 1024, 16, 64
NCORES = 8
HPG = 8            # heads per core
DL = HPG * HD      # 512 local channel dims per core
P = 128

F32 = mybir.dt.float32
BF16 = mybir.dt.bfloat16
EXP = mybir.ActivationFunctionType.Exp

PV = BF16
MMDT = BF16


def _build_program():
    nc = bacc.Bacc("TRN2", target_bir_lowering=False)

    xT = nc.dram_tensor("xT", [C, T], MMDT, kind="ExternalInput").ap()
    # host-pretiled: [p, cc, n] so each SBUF tile is one contiguous DMA
    wv = nc.dram_tensor("wv", [P, 8, DL], MMDT, kind="ExternalInput").ap()
    wq = nc.dram_tensor("wq", [4, P, 8, P], MMDT, kind="ExternalInput").ap()
    wk = nc.dram_tensor("wk", [4, P, 8, P], MMDT, kind="ExternalInput").ap()
    wp = nc.dram_tensor("wp", [P, 4, C], PV, kind="ExternalInput").ap()
    outp = nc.dram_tensor("outp", [T, C], PV, kind="ExternalOutput").ap()

    with tile.TileContext(nc) as tc:
        with ExitStack() as ctx:
            consts = ctx.enter_context(tc.tile_pool(name="consts", bufs=1))
            xt_pool = ctx.enter_context(tc.tile_pool(name="xt", bufs=8))
            w_pool = ctx.enter_context(tc.tile_pool(name="w", bufs=1))
            qk_pool = ctx.enter_context(tc.tile_pool(name="qk", bufs=2))
            v_pool = ctx.enter_context(tc.tile_pool(name="v", bufs=8))
            pt_pool = ctx.enter_context(tc.tile_pool(name="pt", bufs=52))
            yt_pool = ctx.enter_context(tc.tile_pool(name="yt", bufs=8))
            d_pool = ctx.enter_context(tc.tile_pool(name="d", bufs=12))
            out_pool = ctx.enter_context(tc.tile_pool(name="out", bufs=8))
            ps = ctx.enter_context(tc.tile_pool(name="ps", bufs=5, space="PSUM"))
            ps_py = ctx.enter_context(tc.tile_pool(name="psy", bufs=3, space="PSUM"))

            # ---- tiles ----
            xt = [xt_pool.tile([P, T], MMDT, name="xt") for _ in range(8)]
            wv_sb = w_pool.tile([P, 8, DL], MMDT, name="wv")
            wq_sb = [w_pool.tile([P, 8, P], MMDT, name="wq") for _ in range(4)]
            wk_sb = [w_pool.tile([P, 8, P], MMDT, name="wk") for _ in range(4)]
            wp_sb = w_pool.tile([P, 4, C], PV, name="wp")

            # ---- all input DMAs, just-in-time order ----
            # first chunks split fine + on the second HWDGE queue so the
            # v-projection can start as early as possible
            nc.scalar.dma_start(wv_sb[:, 0, :], wv[:, 0, :])
            nc.scalar.dma_start(xt[0][:, 0:256], xT[0:P, 0:256])
            nc.scalar.dma_start(xt[0][:, 256:512], xT[0:P, 256:512])
            nc.scalar.dma_start(xt[0][:, 512:1024], xT[0:P, 512:1024])
            nc.scalar.dma_start(wv_sb[:, 1, :], wv[:, 1, :])
            nc.scalar.dma_start(xt[1][:], xT[P : 2 * P, :])
            for cc in range(2, 8):
                nc.sync.dma_start(xt[cc][:], xT[P * cc : P * (cc + 1), :])
                nc.sync.dma_start(wv_sb[:, cc, :], wv[:, cc, :])
            nc.sync.dma_start(wq_sb[0][:], wq[0])
            nc.sync.dma_start(wk_sb[0][:], wk[0])
            nc.sync.dma_start(wp_sb[:], wp)
            for dt_ in range(1, 4):
                nc.sync.dma_start(wq_sb[dt_][:], wq[dt_])
                nc.sync.dma_start(wk_sb[dt_][:], wk[dt_])

            # ---- constants ----
            tri = consts.tile([P, P], PV, name="tri")  # 1 where tq >= s
            make_upper_triangular(nc, tri[:], val=1.0, diag=True)

            # PE warm-up: dummy matmuls on a zeroed tile while the first x
            # chunks stream in, so the PE clock is ramped to max before the
            # real v-projection starts (cold PE runs 2-4x slower for ~3us)
            warm = consts.tile([P, 512], PV, name="warm")
            nc.vector.memset(warm[:], 0.0)
            wps = ps.tile([P, 512], F32, name="ps")
            for _ in range(24):
                nc.tensor.matmul(
                    wps[:, 0:256],
                    lhsT=warm[:, 0:P],
                    rhs=warm[:, 0:256],
                    start=True,
                    stop=True,
                )

            # v tiles: [s=128, 8 heads x (64 dims + ones col)]
            v_sb = []
            for j in range(8):
                vt = v_pool.tile([P, HPG * (HD + 1)], PV, name="v")
                ones_cols = vt[:].rearrange("p (h e) -> p h e", e=HD + 1)[
                    :, :, HD : HD + 1
                ]
                nc.vector.memset(ones_cols, 1.0)
                v_sb.append(vt)

            qT_sb = [qk_pool.tile([P, 4, T], MMDT, name="qT") for _ in range(2)]
            kT_sb = [qk_pool.tile([P, 4, T], MMDT, name="kT") for _ in range(2)]
            # split per (head pair, query half) so late writers never gate
            # early out-proj reads through coarse dependency tracking
            yT_sb = [
                [yt_pool.tile([P, 512], PV, name="yT") for _ in range(2)]
                for _ in range(4)
            ]

            # ---- v projection (needed by every head pair), 2 waves ----
            for wave in range(2):
                ps_t = [ps.tile([P, 512], F32, name="ps") for _ in range(4)]
                for cc in range(8):
                    for wt_ in range(4):
                        tt = 4 * wave + wt_
                        nc.tensor.matmul(
                            ps_t[wt_][:, :],
                            lhsT=xt[cc][:, P * tt : P * (tt + 1)],
                            rhs=wv_sb[:, cc, :],
                            start=(cc == 0),
                            stop=(cc == 7),
                        )
                for wt_ in range(4):
                    tt = 4 * wave + wt_
                    out_ap = v_sb[tt][:].rearrange("p (h e) -> p h e", e=HD + 1)[
                        :, :, 0:HD
                    ]
                    in_ap = ps_t[wt_][:].rearrange("p (h e) -> p h e", e=HD)
                    nc.scalar.copy(out_ap, in_ap)

            def issue_qk(dt_):
                """q and k projections for head pair dt_ (128 channel dims)."""
                buf = dt_ % 2
                for w_sb, dest, cengs in (
                    (wq_sb[dt_], qT_sb[buf],
                     (nc.vector.tensor_copy, nc.vector.tensor_copy)),
                    (wk_sb[dt_], kT_sb[buf],
                     (nc.vector.tensor_copy, nc.scalar.copy)),
                ):
                    pst = [ps.tile([P, 512], F32, name="ps") for _ in range(2)]
                    for cc in range(8):
                        for tch in range(2):
                            nc.tensor.matmul(
                                pst[tch][:, :],
                                lhsT=w_sb[:, cc, :],
                                rhs=xt[cc][:, 512 * tch : 512 * (tch + 1)],
                                start=(cc == 0),
                                stop=(cc == 7),
                            )
                    for tch in range(2):
                        cengs[tch](
                            dest[:, dt_, 512 * tch : 512 * (tch + 1)], pst[tch][:, :]
                        )

            def emit_st_unit(hp, pts, c, j):
                """One S^T block (both halves) + exp + causal mask."""
                buf = hp % 2
                off = max(0, P * (j - 4 * c))
                n = 512 - off
                for half in range(2):
                    pr = 64 * half
                    pss = ps.tile([P, 512], F32, name="ps")
                    nc.tensor.matmul(
                        pss[:, :n],
                        lhsT=kT_sb[buf][pr : pr + 64, hp, P * j : P * (j + 1)],
                        rhs=qT_sb[buf][pr : pr + 64, hp, 512 * c + off : 512 * (c + 1)],
                        start=True,
                        stop=True,
                    )
                    pt = pt_pool.tile([P, 512], PV, name="pt")
                    nc.scalar.activation(
                        out=pt[:, off:512], in_=pss[:, :n], func=EXP, scale=0.125
                    )
                    if j >= 4 * c:
                        nc.vector.tensor_mul(
                            pt[:, off : off + P], pt[:, off : off + P], tri[:]
                        )
                    pts[(half, c, j)] = pt

            def emit_v_matmul(hp, pts, half, c):
                """V matmuls + D-row extract + reciprocal for one (half, c).

                DVE copies y'+D out of PSUM (freeing the bank), a sync-queue
                DMA remaps the D row to partition 0, DVE reciprocals it there
                (custom op, base-0 only). The bcast+mul live in emit_v_norm so
                recips of later groups never queue behind muls of earlier ones
                on DVE.
                """
                h = 2 * hp + half
                jmax = 4 * c + 3
                py = ps_py.tile([P, 512], F32, name="py")
                for j in range(jmax + 1):
                    off = max(0, P * (j - 4 * c))
                    nc.tensor.matmul(
                        py[0 : HD + 1, off:512],
                        lhsT=v_sb[j][:, (HD + 1) * h : (HD + 1) * (h + 1)],
                        rhs=pts[(half, c, j)][:, off:512],
                        start=(j == 0),
                        stop=(j == jmax),
                    )
                yc = d_pool.tile([HD + 1, 512], F32, name="yc")
                nc.vector.tensor_copy(yc[:], py[0 : HD + 1, :])
                d2 = d_pool.tile([P, 512], F32, name="d")
                nc.sync.dma_start(d2[0:1, :], yc[HD : HD + 1, :])
                # custom DVE op only at partition base 0 (HW quirk)
                nc.vector.reciprocal_approx_fast(d2[0:1, :], d2[0:1, :])
                return yc, d2

            def emit_v_norm(hp, pyd2, half, c):
                """Broadcast 1/D and multiply into yT (odd halves staged)."""
                yc, d2 = pyd2
                nc.gpsimd.partition_broadcast(d2[0:HD, :], d2[0:1, :])
                pr = 64 * half
                dst = yT_sb[hp][c][pr : pr + HD, :]
                if half == 0:
                    nc.vector.tensor_mul(dst, yc[0:HD, :], d2[0:HD, :])
                else:
                    stg = d_pool.tile([HD, 512], PV, name="stg")
                    nc.vector.tensor_mul(stg[:], yc[0:HD, :], d2[0:HD, :])
                    nc.sync.dma_start(dst, stg[:])

            def emit_out_proj(tts):
                for tt in tts:
                    pool, pnm = (ps, "ps") if tt % 2 == 0 else (ps_py, "py")
                    pouts = [pool.tile([P, 512], F32, name=pnm) for _ in range(2)]
                    for dc in range(4):
                        for cch in range(2):
                            nc.tensor.matmul(
                                pouts[cch][:, :],
                                lhsT=yT_sb[dc][tt // 4][
                                    :, P * (tt % 4) : P * (tt % 4 + 1)
                                ],
                                rhs=wp_sb[:, dc, 512 * cch : 512 * (cch + 1)],
                                start=(dc == 0),
                                stop=(dc == 3),
                            )
                    for cch in range(2):
                        ot = out_pool.tile([P, 512], PV, name="out")
                        if cch == 0:
                            nc.scalar.copy(ot[:], pouts[cch][:])
                        else:
                            nc.vector.tensor_copy(ot[:], pouts[cch][:])
                        nc.scalar.dma_start(
                            outp[P * tt : P * (tt + 1), 512 * cch : 512 * (cch + 1)],
                            ot[:],
                        )

            # ---- pipelined qk + attention ----
            # Per head pair: qk proj for the next pair, ALL 12 S^T units for
            # the next pair, then this pair's 4 V groups (c=0 halves first).
            # st units precede v groups on every engine stream, so the
            # normalization chain never blocks the S->PV critical path.
            st_order = [(c, j) for c in range(2) for j in range(4 * c + 4)]
            v_order = [(0, 0), (1, 0), (0, 1), (1, 1)]  # c=0 halves first
            issue_qk(0)
            pts_cur = {}
            for c, j in st_order:
                emit_st_unit(0, pts_cur, c, j)
            for hp in range(4):
                pts_next = {}
                if hp + 1 < 4:
                    issue_qk(hp + 1)
                    pyd = [emit_v_matmul(hp, pts_cur, *v_order[0])]
                    pyd.append(emit_v_matmul(hp, pts_cur, *v_order[1]))
                    for c, j in st_order:
                        emit_st_unit(hp + 1, pts_next, c, j)
                    emit_v_norm(hp, pyd[0], *v_order[0])
                    pyd.append(emit_v_matmul(hp, pts_cur, *v_order[2]))
                    emit_v_norm(hp, pyd[1], *v_order[1])
                    pyd.append(emit_v_matmul(hp, pts_cur, *v_order[3]))
                    emit_v_norm(hp, pyd[2], *v_order[2])
                    emit_v_norm(hp, pyd[3], *v_order[3])
                else:
                    pyd = [emit_v_matmul(hp, pts_cur, *v_order[0])]
                    for g in range(1, 4):
                        pyd.append(emit_v_matmul(hp, pts_cur, *v_order[g]))
                        emit_v_norm(hp, pyd[g - 1], *v_order[g - 1])
                    emit_v_norm(hp, pyd[3], *v_order[3])
                    emit_out_proj(range(8))
                pts_cur = pts_next

    nc.compile()
    return nc


_CACHED_NC = None


def _get_program():
    global _CACHED_NC
    if _CACHED_NC is None:
        _CACHED_NC = _build_program()
    return _CACHED_NC


def _prepare_in_maps(x, W_qkv, b_qkv, W_proj):
    x = np.asarray(x, np.float32)
    W_qkv = np.asarray(W_qkv, np.float32)
    W_proj = np.asarray(W_proj, np.float32)
    mm_np = ml_dtypes.bfloat16

    in_maps = []
    for core in range(NCORES):
        b, hg = core // 2, core % 2
        lo = hg * DL
        # [C, DL] -> [4dt|8cc, 128p, ...] pretiled so SBUF tiles are contiguous
        wq_s = W_qkv[:, lo : lo + DL].astype(mm_np)
        wk_s = W_qkv[:, C + lo : C + lo + DL].astype(mm_np)
        wv_s = W_qkv[:, 2 * C + lo : 2 * C + lo + DL].astype(mm_np)
        # wv: [1024, 512] -> [8cc, 128p, 512] -> [128p, 8cc, 512]
        wv_t = np.ascontiguousarray(wv_s.reshape(8, P, DL).transpose(1, 0, 2))
        # wq/wk: [1024, 512] -> [8cc, 128p, 4dt, 128n] -> [4dt, 128p, 8cc, 128n]
        wq_t = np.ascontiguousarray(
            wq_s.reshape(8, P, 4, P).transpose(2, 1, 0, 3)
        )
        wk_t = np.ascontiguousarray(
            wk_s.reshape(8, P, 4, P).transpose(2, 1, 0, 3)
        )
        # wp: [512, 1024] -> [4dc, 128p, 1024] -> [128p, 4dc, 1024]
        wp_s = W_proj[lo : lo + DL, :].astype(mm_np)
        wp_t = np.ascontiguousarray(wp_s.reshape(4, P, C).transpose(1, 0, 2))
        in_maps.append(
            {
                "xT": np.ascontiguousarray(x[b].T).astype(mm_np),
                "wq": wq_t,
                "wk": wk_t,
                "wv": wv_t,
                "wp": wp_t,
            }
        )
    return in_maps


def _reference_fallback(x, W_qkv, b_qkv, W_proj, b_proj):
    """Numpy reference path; only taken if b_qkv is unexpectedly nonzero."""
    x = np.asarray(x, np.float32)
    qkv = x @ np.asarray(W_qkv, np.float32) + np.asarray(b_qkv, np.float32)
    q, k, v = np.split(qkv, 3, axis=-1)

    def heads(t):
        return t.reshape(B, T, H, HD).transpose(0, 2, 1, 3)

    q, k, v = heads(q), heads(k), heads(v)
    s = np.einsum("bhqd,bhkd->bhqk", q, k) / np.sqrt(np.float32(HD))
    s = np.where(np.tril(np.ones((T, T), bool)), s, -np.inf)
    s -= s.max(axis=-1, keepdims=True)
    p = np.exp(s)
    p /= p.sum(axis=-1, keepdims=True)
    y = np.einsum("bhqk,bhkd->bhqd", p, v)
    y = y.transpose(0, 2, 1, 3).reshape(B, T, C)
    return y @ np.asarray(W_proj, np.float32) + np.asarray(b_proj, np.float32)


def run(inputs, trace=False):
    nc = _get_program()
    in_maps = _prepare_in_maps(
        inputs["x"], inputs["W_qkv"], inputs["b_qkv"], inputs["W_proj"]
    )
    res = run_bass_kernel_spmd(nc, in_maps, core_ids=list(range(NCORES)), trace=trace)
    b_proj = np.asarray(inputs["b_proj"], np.float32)
    out = np.empty((B, T, C), np.float32)
    for b in range(B):
        out[b] = (
            res.results[2 * b]["outp"].astype(np.float32)
            + res.results[2 * b + 1]["outp"].astype(np.float32)
            + b_proj
        )
    return out, res


def kernel(**inputs):
    if np.any(np.asarray(inputs["b_qkv"], np.float32)):
        return _reference_fallback(
            inputs["x"],
            inputs["W_qkv"],
            inputs["b_qkv"],
            inputs["W_proj"],
            inputs["b_proj"],
        )
    out, _ = run(inputs, trace=False)
    return out
